# revision 1
# baseline (speedup 1.0000x reference)
"""Depth rasterization (MANO hand z-buffer @ 640x640 -> bilinear 128x128).

Key identities exploited:
  * jax.image.resize(640->128, linear, antialias=False) samples input coords
    5*j + 2.0 exactly -> output[i, j] == raster[5i+2, 5j+2]. Only the 128x128
    decimated pixel grid (centers x = 5j+2.5, y = 5i+2.5) is rasterized: a
    25x reduction vs the reference's 640x640 raster.
  * Edge functions and barycentric depth are affine in pixel coords, so each
    triangle yields four planes over the basis (j, i, 1):
      P_k = OFF - S * sign(area) * e_k     (k = 0,1,2 penalty planes)
      W   = (e0*z0 + e1*z1 + e2*z2) / area (depth plane)
    key(p, f) = max(P0, P1, P2, W) equals the interpolated depth when p is
    inside triangle f and is >= OFF (>> the 100 clamp) outside; the z-buffer
    is zbuf(p) = min(100, min_f key(p, f)).
  * Plane evaluation is a K=9 bf16 matmul (coefficients split into 3 bf16
    limbs; the (j, i, 1) basis is exact in bf16, giving fp32-grade accuracy
    at bf16 PE speed); planes are pair-merged as comp-A = [P0|W] and
    comp-B = [P1|P2] streams evaluated on alternating PE row-groups.
  * Per 16x8-pixel tile, candidates are bbox-filtered and hierarchical-z
    pruned on the host (exact: a candidate whose minimum possible depth over
    the tile exceeds the best fully-covering candidate's maximum depth can
    never win). Tiles are chunked to <=256 candidates per work item (host
    min-merges chunks), items are rank-parity balanced across each batch's
    two cores, and slot capacities are per-rank maxima across all 8 cores -
    exact for any input, no truncation.
  * DVE work per slot is 3 element passes: one wide tensor_tensor max
    (u = max(compA, compB)) and a custom fused DVE op
    (out = max(u_lo, u_hi); accum = min-reduce seeded at 100).

Sharding: 8 cores; each batch element's 128 tiles split across 2 cores.
"""

import numpy as np
import ml_dtypes

import concourse.bacc as bacc
import concourse.mybir as mybir
import concourse.tile as tile
from concourse.bass_utils import run_bass_kernel_spmd

_B, _V, _F = 4, 778, 1538
_H = _W = 128
_TJ, _TI = 16, 8   # tile size in output pixels (x, y)
_NTILE = (_H // _TI) * (_W // _TJ)  # 128 tiles per batch image
_WMAX = 256        # max slot width (pair-merged 2w <= 512 = one PSUM bank)
_OFF = 1000.0      # penalty-plane offset (>> 100 clamp)
_S = 1.0e9         # penalty scale
_BIGC = 1.0e7      # plane constant for padding/invalid
_CLAMP = 100.0
_COVER_MARGIN = 1.0    # e*s margin (e-units) for the full-cover test
_BOUND_MARGIN = 1e-3   # depth margin for the prune bound

_F32 = mybir.dt.float32
_BF16 = mybir.dt.bfloat16
_BF16_NP = ml_dtypes.bfloat16

_NC_CACHE = {}
_OP_CACHE = {}
PROFILE = {}


def _maxpair_minred_op():
    """Custom DVE op: out = max(in0, in1); accum_out = min(out) seeded s0."""
    if "op" in _OP_CACHE:
        return _OP_CACHE["op"]
    import concourse.dve_ops as dve_ops
    from concourse.dve_spec import C0, Spec, Src0, Src1, lower, maxx, minn
    from concourse.dve_table_gen import dve_ver_for
    from concourse.dve_uop import DveOpSpec

    name = "MAXPAIR_MINRED_ANT"
    for op in dve_ops.OPS:
        if op.name == name:
            _OP_CACHE["op"] = op
            return op
    spec = Spec(body=maxx(Src0, Src1), accum=minn, accum_init=C0)
    opcode = dve_ops._CUSTOM_DVE_ROW_BASE + len(dve_ops.OPS)
    assert opcode < 0x20
    dve_ops._SUB_OPCODE_FOR_NAME[name] = opcode
    ver = dve_ver_for("TRN2")
    sha = DveOpSpec(name=name, opcode=opcode, uops=lower(spec, ver=ver),
                    rd1_en=True).sha(ver)
    op = dve_ops.DveOp(name, spec, subdim=False, uops_sha={ver: sha})
    dve_ops.OPS.append(op)
    dve_ops.CUSTOM_DVE_SPECS[name] = spec
    _OP_CACHE["op"] = op
    return op


def _build_nc(caps, groups):
    """caps: per-slot widths w (32-granular, <= _WMAX); groups: ((w, k), ...)
    of consecutive equal-width slots with 2*k*w <= 512 (one PSUM bank)."""
    nslot = len(caps)
    total2 = 2 * int(sum(caps))
    op = _maxpair_minred_op()
    nc = bacc.Bacc("TRN2", target_bir_lowering=False, debug=False, num_devices=8)
    # dense [128, ...] input: pair-merged coef streams (comp-A = [P0|W] limbs
    # at partitions 0-8 & 64-72, comp-B = [P1|P2] at 32-40 & 96-104), then
    # nslot*128 pixel-basis cols at all four row-groups.
    data_d = nc.dram_tensor("data", [128, total2 + nslot * 128], _BF16, kind="ExternalInput")
    out_d = nc.dram_tensor("out", [128, nslot], _F32, kind="ExternalOutput")

    with tile.TileContext(nc) as tc:
        with (
            tc.tile_pool(name="const", bufs=1) as cpool,
            tc.tile_pool(name="scr", bufs=6) as spool,
            tc.tile_pool(name="ps", bufs=8, space="PSUM") as ppool,
        ):
            zmin = cpool.tile([128, nslot], _F32)
            # coef DMA in ~6 chunks at group boundaries; pix in 4 chunks
            goff = [0]
            for w, k in groups:
                goff.append(goff[-1] + 2 * w * k)
            # chunk boundaries (in groups): fine-grained early so the first
            # compute groups start as soon as their data lands
            gb = [0, 1, 2, 4, 6, 9, 13, 18, 24]
            gb = sorted({min(g, len(groups)) for g in gb} | {len(groups)})
            slot_of_group = [0]
            for w, k in groups:
                slot_of_group.append(slot_of_group[-1] + k)
            ctiles = []  # (col range, tile)
            ptiles = []  # (slot range, tile)
            dmas = []
            for i in range(len(gb) - 1):
                c0, c1 = goff[gb[i]], goff[gb[i + 1]]
                s0, s1 = slot_of_group[gb[i]], slot_of_group[gb[i + 1]]
                if c1 > c0:
                    ct = cpool.tile([128, c1 - c0], _BF16, name=f"coef{i}")
                    ctiles.append((c0, c1, ct))
                    dmas.append((ct, data_d.ap()[:, c0:c1]))
                if s1 > s0:
                    pt = cpool.tile([128, (s1 - s0) * 128], _BF16, name=f"pix{i}")
                    ptiles.append((s0, s1, pt))
                    dmas.append((pt, data_d.ap()[:, total2 + s0 * 128 : total2 + s1 * 128]))
            for dst, srcap in dmas:
                nc.sync.dma_start(dst[:], srcap)

            def coef_view(c0, c1):
                for t0, t1, ct in ctiles:
                    if t0 <= c0 and c1 <= t1:
                        return ct[:, c0 - t0 : c1 - t0]
                raise AssertionError((c0, c1))

            def pix_view(s):
                for s0, s1, pt in ptiles:
                    if s0 <= s < s1:
                        return pt[:, (s - s0) * 128 : (s - s0 + 1) * 128]
                raise AssertionError(s)

            gbase = 0
            for gi, (w, k) in enumerate(groups):
                kw2 = 2 * w * k
                go = goff[gi]
                pa = ppool.tile([128, 512], _F32, tag="ps", name="pa")
                pb = ppool.tile([128, 512], _F32, tag="ps", name="pb")
                for q in range(k):
                    s = gbase + q
                    o = 2 * w * q
                    ra, rb = (0, 32) if gi % 2 == 0 else (64, 96)
                    pv = pix_view(s)
                    cv = coef_view(go + o, go + o + 2 * w)
                    nc.tensor.matmul(pa[:, o : o + 2 * w], pv[ra : ra + 9, :],
                                     cv[ra : ra + 9, :],
                                     start=True, stop=True, tile_position=(ra, 0))
                    nc.tensor.matmul(pb[:, o : o + 2 * w], pv[rb : rb + 9, :],
                                     cv[rb : rb + 9, :],
                                     start=True, stop=True, tile_position=(rb, 0))
                # ScalarE pulls comp-A to SBUF (DVE reads max one PSUM operand)
                ta = spool.tile([128, 512], _F32, tag="ta", name="ta")
                nc.scalar.copy(ta[:, :kw2], pa[:, :kw2])
                u = spool.tile([128, 512], _F32, tag="u", name="u")
                nc.vector.tensor_tensor(u[:, :kw2], ta[:, :kw2], pb[:, :kw2],
                                        op=mybir.AluOpType.max)
                for q in range(k):
                    s = gbase + q
                    o = 2 * w * q
                    keyt = spool.tile([128, 256], _F32, tag="key", name="keyt")
                    if PROFILE.get("no_custom"):
                        nc.vector.tensor_tensor(keyt[:, :w], u[:, o : o + w],
                                                u[:, o + w : o + 2 * w],
                                                op=mybir.AluOpType.max)
                        nc.vector.tensor_reduce(zmin[:, s : s + 1], keyt[:, :w],
                                                axis=mybir.AxisListType.X,
                                                op=mybir.AluOpType.min)
                    else:
                        nc.vector._custom_dve(
                            op,
                            out=keyt[:, :w],
                            in0=u[:, o : o + w],
                            in1=u[:, o + w : o + 2 * w],
                            s0=_CLAMP,
                            accum_out=zmin[:, s : s + 1],
                        )
                gbase += k

            nc.sync.dma_start(out_d.ap(), zmin[:])

    nc.compile()
    return nc


def _get_nc(caps, groups):
    key = (caps, groups)
    if key not in _NC_CACHE:
        _NC_CACHE[key] = _build_nc(caps, groups)
    return _NC_CACHE[key]


def _planes64(vertices, faces):
    """Full-precision planes on basis (j, i, 1): [B, 4, 3, F] f64 + aux."""
    v64 = vertices.astype(np.float64)
    fidx = np.asarray(faces).astype(np.int64).reshape(-1)
    fv = v64[:, fidx, :].reshape(_B, _F, 3, 3)
    x0, y0, z0 = fv[:, :, 0, 0], fv[:, :, 0, 1], fv[:, :, 0, 2]
    x1, y1, z1 = fv[:, :, 1, 0], fv[:, :, 1, 1], fv[:, :, 1, 2]
    x2, y2, z2 = fv[:, :, 2, 0], fv[:, :, 2, 1], fv[:, :, 2, 2]

    # area exactly as the reference computes it (float32 ops)
    v32 = vertices.astype(np.float32)
    fv32 = v32[:, fidx, :].reshape(_B, _F, 3, 3)
    xa, ya = fv32[:, :, 0, 0], fv32[:, :, 0, 1]
    xb, yb = fv32[:, :, 1, 0], fv32[:, :, 1, 1]
    xc, yc = fv32[:, :, 2, 0], fv32[:, :, 2, 1]
    area32 = (xb - xa) * (yc - ya) - (yb - ya) * (xc - xa)
    s = np.sign(area32).astype(np.float64)
    valid = np.abs(area32) > 1e-12

    A0 = -(y2 - y1); B0 = x2 - x1; C0 = (y2 - y1) * x1 - (x2 - x1) * y1
    A1 = -(y0 - y2); B1 = x0 - x2; C1 = (y0 - y2) * x2 - (x0 - x2) * y2
    A2 = -(y1 - y0); B2 = x1 - x0; C2 = (y1 - y0) * x0 - (x1 - x0) * y0

    area64 = np.where(valid, area32.astype(np.float64), 1.0)
    Aw = (z0 * A0 + z1 * A1 + z2 * A2) / area64
    Bw = (z0 * B0 + z1 * B1 + z2 * B2) / area64
    Cw = (z0 * C0 + z1 * C1 + z2 * C2) / area64

    planes = np.zeros((_B, 4, 3, _F), np.float64)
    raw = [
        (-_S * s * A0, -_S * s * B0, _OFF - _S * s * C0),
        (-_S * s * A1, -_S * s * B1, _OFF - _S * s * C1),
        (-_S * s * A2, -_S * s * B2, _OFF - _S * s * C2),
        (Aw, Bw, Cw),
    ]
    for k, (a, b, c) in enumerate(raw):
        a = np.where(valid, a, 0.0)
        b = np.where(valid, b, 0.0)
        c = np.where(valid, c, _BIGC)
        # basis change px = 5j + 2.5, py = 5i + 2.5 -> (j, i, 1)
        planes[:, k, 0] = 5.0 * a
        planes[:, k, 1] = 5.0 * b
        planes[:, k, 2] = 2.5 * a + 2.5 * b + c

    xsmin = fv[..., 0].min(2); xsmax = fv[..., 0].max(2)
    ysmin = fv[..., 1].min(2); ysmax = fv[..., 1].max(2)
    zmin_tri = fv[..., 2].min(2)
    return planes, valid, xsmin, xsmax, ysmin, ysmax, zmin_tri


def _split3(c64):
    hi = c64.astype(_BF16_NP).astype(np.float64)
    mid = (c64 - hi).astype(_BF16_NP).astype(np.float64)
    lo = (c64 - hi - mid).astype(_BF16_NP)
    return hi.astype(_BF16_NP), mid.astype(_BF16_NP), lo


def _prepare(vertices, faces):
    planes, valid, xsmin, xsmax, ysmin, ysmax, zmin_tri = _planes64(vertices, faces)
    ntj = _W // _TJ

    # prune per tile, chunk to <=_WMAX, rank-parity balance across all 8
    # cores (a core may hold tiles of any batch - the coef stream is data)
    core_items = [[] for _ in range(8)]  # items: (batch, tile_t, cand_idx_array)
    all_items = []
    for b in range(_B):
        P = planes[b]
        items = all_items
        for t in range(_NTILE):
            tj, ti = t % ntj, t // ntj
            j0, i0 = tj * _TJ, ti * _TI
            xlo, xhi = 5 * j0 + 2.5, 5 * (j0 + _TJ - 1) + 2.5
            ylo, yhi = 5 * i0 + 2.5, 5 * (i0 + _TI - 1) + 2.5
            cand = np.where(valid[b] & (xsmax[b] >= xlo) & (xsmin[b] <= xhi)
                            & (ysmax[b] >= ylo) & (ysmin[b] <= yhi))[0]
            if len(cand):
                corners = np.array(
                    [[j0, i0, 1], [j0 + _TJ - 1, i0, 1],
                     [j0, i0 + _TI - 1, 1], [j0 + _TJ - 1, i0 + _TI - 1, 1]],
                    np.float64)
                Wc = corners @ P[3][:, cand]
                zlo = np.maximum(Wc.min(0), zmin_tri[b][cand])
                covers = np.ones(len(cand), bool)
                for k in range(3):
                    Pc = corners @ P[k][:, cand]
                    covers &= (Pc <= _OFF - _S * _COVER_MARGIN).all(axis=0)
                bound = (Wc.max(0)[covers].min() + _BOUND_MARGIN) if covers.any() else np.inf
                keep = zlo <= bound
                order = cand[keep][np.argsort(zlo[keep])]
            else:
                order = cand
            if len(order) == 0:
                items.append((b, t, order))
            else:
                for c0 in range(0, len(order), _WMAX):
                    items.append((b, t, order[c0 : c0 + _WMAX]))
    all_items.sort(key=lambda it: -len(it[2]))
    for r, it in enumerate(all_items):
        core_items[r % 8].append(it)

    nslot = max(len(ci) for ci in core_items)
    rawcaps = []
    for s in range(nslot):
        m = max((len(ci[s][2]) if s < len(ci) else 0) for ci in core_items)
        rawcaps.append(max(16, ((m + 15) // 16) * 16))

    # groups of consecutive slots padded to the group's (max) width, with
    # pair-merged group width 2*k*w <= 512 (one PSUM bank)
    groups = []
    s = 0
    while s < nslot:
        w = rawcaps[s]
        k = 1
        while s + k < nslot and 2 * (k + 1) * w <= 512:
            k += 1
        groups.append((w, k))
        s += k
    groups = tuple(groups)
    caps = []
    for w, k in groups:
        caps.extend([w] * k)
    caps = tuple(caps)
    total2 = 2 * sum(caps)

    in_maps = []
    for c in range(8):
        items = core_items[c]
        compA = np.zeros((3, total2), np.float64)
        compB = np.zeros((3, total2), np.float64)
        compA[2, :] = _BIGC
        compB[2, :] = _BIGC
        pix_g = np.zeros((3, nslot * 128), np.float32)
        off = 0
        for s in range(nslot):
            w = caps[s]
            jj = ii = np.zeros(128, np.float32)
            if s < len(items):
                b, t, idx = items[s]
                n = len(idx)
                compA[:, off : off + n] = planes[b, 0][:, idx]          # P0
                compA[:, off + w : off + w + n] = planes[b, 3][:, idx]  # W
                compB[:, off : off + n] = planes[b, 1][:, idx]          # P1
                compB[:, off + w : off + w + n] = planes[b, 2][:, idx]  # P2
                tj, ti = t % ntj, t // ntj
                j0, i0 = tj * _TJ, ti * _TI
                jj = j0 + np.tile(np.arange(_TJ, dtype=np.float32), _TI)
                ii = i0 + np.repeat(np.arange(_TI, dtype=np.float32), _TJ)
            off += 2 * w
            pix_g[0, s * 128 : (s + 1) * 128] = jj
            pix_g[1, s * 128 : (s + 1) * 128] = ii
            pix_g[2, s * 128 : (s + 1) * 128] = 1.0
        data = np.zeros((128, total2 + nslot * 128), _BF16_NP)
        for comp, bases in ((compA, (0, 64)), (compB, (32, 96))):
            hi, mid, lo = _split3(comp)
            for base in bases:
                data[base + 0 : base + 3, :total2] = hi
                data[base + 3 : base + 6, :total2] = mid
                data[base + 6 : base + 9, :total2] = lo
        pix16 = np.vstack([pix_g, pix_g, pix_g]).astype(_BF16_NP)
        for base in (0, 32, 64, 96):
            data[base : base + 9, total2:] = pix16
        in_maps.append({"data": data})
    return caps, groups, in_maps, core_items


def kernel(vertices, faces):
    vertices = np.asarray(vertices)
    faces = np.asarray(faces)
    caps, groups, in_maps, core_items = _prepare(vertices, faces)

    nc = _get_nc(caps, groups)
    kw = dict(PROFILE.get("run_kwargs", {}))
    res = run_bass_kernel_spmd(nc, in_maps, list(range(8)), **kw)
    PROFILE["last_result"] = res

    ntj = _W // _TJ
    out = np.full((_B, _H, _W), _CLAMP, np.float32)
    for c in range(8):
        z = res.results[c]["out"]  # [128, nslot]
        for s, (b, t, idx) in enumerate(core_items[c]):
            tj, ti = t % ntj, t // ntj
            j0, i0 = tj * _TJ, ti * _TI
            blk = z[:, s].reshape(_TI, _TJ)
            out[b, i0 : i0 + _TI, j0 : j0 + _TJ] = np.minimum(
                out[b, i0 : i0 + _TI, j0 : j0 + _TJ], blk)
    return out



# revision 2
# speedup vs baseline: 1.0236x; 1.0236x over previous
"""Depth rasterization v2 (MANO z-buffer 640x640 -> bilinear 128x128).

Identities (see v1): output[i,j] == raster at pixel centers (5j+2.5, 5i+2.5);
per (tile, triangle) the inside-test + depth is key = max(penalty planes, W)
with planes affine in pixel coords; zbuf = min(100, min keys).

v2 structure:
  * 4x4-output-px tiles (16 px). g=8 tiles stack into one matmul column set:
    PE stationary is a block-diagonal [72, 128] basis (9 rows x 8 partition
    blocks of 16 px), so each moving column evaluates 8 tiles' candidates at
    once -> ~8x fewer columns everywhere.
  * Host pruning per tile: edge-cull (corner max e*s < -margin), 2x2-px
    subrect hierarchical-z (corner-exact), conservative margins vs the
    reference's f32 arithmetic.
  * Classes by active edges over the tile: c0 (cover) -> W only, 1 PSUM col,
    min-reduced straight from PSUM; c1 (1 active edge) -> [W | Pa] pair;
    cw (2-3 active) -> A=[Pa|W], B=[Pb|Pc] pair-merged max tree.
  * Engines: PE matmuls -> ACT copies A-side PSUM->SBUF bf16 -> DVE does
    max-combine + segmented X-reduces -> one DMA out. ~30 instructions/core.
"""

import numpy as np
import ml_dtypes

import concourse.bacc as bacc
import concourse.mybir as mybir
import concourse.tile as tile
from concourse.bass_utils import run_bass_kernel_spmd

_B, _V, _F = 4, 778, 1538
_HW = 128
_T = 4            # tile edge in output px (4x4)
_NTJ = _HW // _T  # 32 tiles per row
_G = 8            # tiles stacked per column (partition blocks of 16 px)
_PPT = 128 // _G  # 16 px per tile
_K = 9 * _G       # contraction: 3 limbs x 3 basis x 8 blocks
_OFF = 1000.0
_S = 1.0e9
_BIGC = 1.0e7
_CLAMP = 100.0
_MARGIN = 1.0     # e-units: host f64 vs ref f32 + device noise
_BOUNDM = 1e-3
_CHUNK = 64       # max cands per (tile, class) slot member

_F32 = mybir.dt.float32
_BF16 = mybir.dt.bfloat16
_BF = ml_dtypes.bfloat16

_NC_CACHE = {}
PROFILE = {}


# ---------------------------------------------------------------- geometry --

def _planes(vertices, faces):
    v64 = vertices.astype(np.float64)
    fidx = np.asarray(faces).astype(np.int64).reshape(-1)
    fv = v64[:, fidx, :].reshape(_B, _F, 3, 3)
    v32 = vertices.astype(np.float32)
    fv32 = v32[:, fidx, :].reshape(_B, _F, 3, 3)
    xa, ya = fv32[:, :, 0, 0], fv32[:, :, 0, 1]
    xb, yb = fv32[:, :, 1, 0], fv32[:, :, 1, 1]
    xc, yc = fv32[:, :, 2, 0], fv32[:, :, 2, 1]
    area32 = (xb - xa) * (yc - ya) - (yb - ya) * (xc - xa)
    s = np.sign(area32).astype(np.float64)
    valid = np.abs(area32) > 1e-12

    x0, y0, z0 = fv[:, :, 0, 0], fv[:, :, 0, 1], fv[:, :, 0, 2]
    x1, y1, z1 = fv[:, :, 1, 0], fv[:, :, 1, 1], fv[:, :, 1, 2]
    x2, y2, z2 = fv[:, :, 2, 0], fv[:, :, 2, 1], fv[:, :, 2, 2]
    A0 = -(y2 - y1); B0 = x2 - x1; C0 = (y2 - y1) * x1 - (x2 - x1) * y1
    A1 = -(y0 - y2); B1 = x0 - x2; C1 = (y0 - y2) * x2 - (x0 - x2) * y2
    A2 = -(y1 - y0); B2 = x1 - x0; C2 = (y1 - y0) * x0 - (x1 - x0) * y0
    area64 = np.where(valid, area32.astype(np.float64), 1.0)
    Aw = (z0 * A0 + z1 * A1 + z2 * A2) / area64
    Bw = (z0 * B0 + z1 * B1 + z2 * B2) / area64
    Cw = (z0 * C0 + z1 * C1 + z2 * C2) / area64
    # e_k * s planes (inside iff >= 0), [3 edge, 3 coef(x,y,1), B, F]
    E = np.stack([np.stack([A0 * s, B0 * s, C0 * s]),
                  np.stack([A1 * s, B1 * s, C1 * s]),
                  np.stack([A2 * s, B2 * s, C2 * s])])
    W = np.stack([Aw, Bw, Cw])  # [3, B, F]
    bbox = (fv[..., 0].min(2), fv[..., 0].max(2),
            fv[..., 1].min(2), fv[..., 1].max(2))
    return E, W, bbox, fv[..., 2].min(2), valid


def _split3(c64):
    hi = c64.astype(_BF).astype(np.float64)
    mid = (c64 - hi).astype(_BF).astype(np.float64)
    lo = (c64 - hi - mid).astype(_BF)
    return hi.astype(_BF), mid.astype(_BF), lo


# ------------------------------------------------------------------- prune --

def _gather_pairs(vertices, faces):
    """Prune + classify. Returns per-class dicts keyed (b, t):
    lists of cand plane data, with classes 0 (W), 1 (W,Pa), 2 (Pa,Pb,Pc,W)."""
    E, W, (xmn, xmx, ymn, ymx), ztri, valid = _planes(vertices, faces)
    ntile = _NTJ * _NTJ
    tj = np.tile(np.arange(_NTJ), _NTJ)
    ti = np.repeat(np.arange(_NTJ), _NTJ)
    j0 = tj * _T
    i0 = ti * _T
    xlo = 5.0 * j0 + 2.5; xhi = 5.0 * (j0 + _T - 1) + 2.5
    ylo = 5.0 * i0 + 2.5; yhi = 5.0 * (i0 + _T - 1) + 2.5

    out = []  # per class: entries (b, t, edges, cand_f)
    for b in range(_B):
        m = (valid[b][None, :]
             & (xmx[b][None, :] >= xlo[:, None]) & (xmn[b][None, :] <= xhi[:, None])
             & (ymx[b][None, :] >= ylo[:, None]) & (ymn[b][None, :] <= yhi[:, None]))
        t_idx, f_idx = np.nonzero(m)
        if len(t_idx) == 0:
            continue
        Eb = E[:, :, b][:, :, f_idx]      # [3, 3, P]
        Wb = W[:, b][:, f_idx]            # [3, P]
        pj0 = 5.0 * (t_idx % _NTJ) * _T + 2.5
        pi0 = 5.0 * (t_idx // _NTJ) * _T + 2.5

        def vals(dx0, dx1, dy0, dy1):
            # plane values at the 4 corners px = pj0+{dx0,dx1}, py = pi0+{dy0,dy1}
            cx = np.stack([pj0 + dx0, pj0 + dx1, pj0 + dx0, pj0 + dx1])  # [4,P]
            cy = np.stack([pi0 + dy0, pi0 + dy0, pi0 + dy1, pi0 + dy1])
            ev = (Eb[:, 0][:, None] * cx[None] + Eb[:, 1][:, None] * cy[None]
                  + Eb[:, 2][:, None])   # [3, 4, P]
            wv = Wb[0][None] * cx + Wb[1][None] * cy + Wb[2][None]  # [4, P]
            return ev, wv

        ev_t, _ = vals(0.0, 5.0 * (_T - 1), 0.0, 5.0 * (_T - 1))
        cull = (ev_t.max(1) < -_MARGIN).any(0)
        act = ev_t.min(1) < _MARGIN      # [3, P] active edges (tile level)

        alive = np.zeros(len(t_idx), bool)
        half = 5.0 * (_T // 2)
        for sj in range(2):
            for si in range(2):
                ev, wv = vals(sj * half, sj * half + 5.0, si * half, si * half + 5.0)
                out_s = (ev.max(1) < -_MARGIN).any(0)
                cov_s = (ev.min(1) >= _MARGIN).all(0)
                zlo_s = np.maximum(wv.min(0), ztri[b][f_idx])
                whi = wv.max(0)
                bnd = np.full(ntile, np.inf)
                lc = cov_s & ~out_s
                np.minimum.at(bnd, t_idx[lc], whi[lc])
                alive |= ~out_s & (zlo_s <= bnd[t_idx] + _BOUNDM)
        keep = ~cull & alive
        nact = act.sum(0)
        cls = np.minimum(nact, 2)
        for c in range(3):
            sel = np.nonzero(keep & (cls == c))[0]
            if len(sel) == 0:
                continue
            out.append((b, c, t_idx[sel], f_idx[sel], act[:, sel]))
    return out


# ------------------------------------------------------------------ packing --

def _prepare(vertices, faces):
    E, W, _, _, _ = _planes(vertices, faces)
    pairs = _gather_pairs(vertices, faces)

    # per (class) -> dict tile-> (b, t, [f...], [act...])
    tl = {0: {}, 1: {}, 2: {}}
    for b, c, t_idx, f_idx, act in pairs:
        order = np.argsort(t_idx, kind='stable')
        t_s, f_s, a_s = t_idx[order], f_idx[order], act[:, order]
        cuts = np.nonzero(np.diff(t_s))[0] + 1
        for tt, ff, aa in zip(np.split(t_s, cuts), np.split(f_s, cuts),
                              np.split(a_s, cuts, axis=1)):
            key = (b, int(tt[0]))
            tl[c][key] = (ff, aa)

    # chunk to <=_CHUNK, build member lists per class
    members = {0: [], 1: [], 2: []}  # (b, t, f_array, act_array)
    for c in range(3):
        for (b, t), (ff, aa) in tl[c].items():
            for o in range(0, len(ff), _CHUNK):
                members[c].append((b, t, ff[o:o + _CHUNK], aa[:, o:o + _CHUNK]))

    # stack g members of similar width; stacks -> round-robin cores by rank
    stacks = {}
    for c in range(3):
        ms = sorted(members[c], key=lambda m: -len(m[2]))
        stacks[c] = [ms[i:i + _G] for i in range(0, len(ms), _G)]

    core_slots = [ {0: [], 1: [], 2: []} for _ in range(8) ]
    caps = {0: [], 1: [], 2: []}
    for c in range(3):
        for r, st in enumerate(stacks[c]):
            core_slots[r % 8][c].append(st)
        nrank = (len(stacks[c]) + 7) // 8
        for rank in range(nrank):
            wmax = 0
            for core in range(8):
                cs = core_slots[core][c]
                if rank < len(cs):
                    wmax = max(wmax, max(len(m[2]) for m in cs[rank]))
            cap = 4
            while cap < wmax:
                cap *= 2
            caps[c].append(cap)  # power-of-2: few equal-cap runs

    cap0, cap1, capw = caps[0], caps[1], caps[2]
    S0, S1, Sw = len(cap0), len(cap1), len(capw)
    T0, T1, Tw = sum(cap0), sum(cap1), sum(capw)
    CA = T0 + T1 + 2 * Tw           # A-side psum cols
    CB = T1 + 2 * Tw                # B-side psum cols
    Stot = S0 + S1 + Sw

    # fill coef streams per core
    in_maps = []
    meta = []  # per core: list of (class, rank, member_idx->(b,t))
    for core in range(8):
        cA = np.zeros((3, _G, CA), np.float64)
        cB = np.zeros((3, _G, CB), np.float64)
        cA[2] = _BIGC
        cB[2] = _BIGC
        core_meta = []
        offA = {0: 0, 1: T0, 2: T0 + T1}
        offB = {1: 0, 2: T1}
        for c, capl in ((0, cap0), (1, cap1), (2, capw)):
            oA = offA[c]
            oB = offB.get(c, 0)
            for rank, cap in enumerate(capl):
                slot = core_slots[core][c][rank] if rank < len(core_slots[core][c]) else []
                slot_meta = []
                for u, (b, t, ff, aa) in enumerate(slot):
                    n = len(ff)
                    tjj = (t % _NTJ) * _T
                    tii = (t // _NTJ) * _T
                    px0 = 5.0 * tjj + 2.5
                    py0 = 5.0 * tii + 2.5
                    Eb = E[:, :, b][:, :, ff]   # [3,3,n]
                    Wb = W[:, b][:, ff]         # [3,n]
                    # folded planes on (dj, di, 1)
                    wa = 5.0 * Wb[0]; wb = 5.0 * Wb[1]
                    wc = Wb[0] * px0 + Wb[1] * py0 + Wb[2]
                    # penalty planes P_k = OFF - S*(e_k*s) on (dj, di, 1)
                    pen = []
                    for k in range(3):
                        pen.append((-5.0 * _S * Eb[k, 0], -5.0 * _S * Eb[k, 1],
                                    _OFF - _S * (Eb[k, 0] * px0 + Eb[k, 1] * py0 + Eb[k, 2])))
                    if c == 0:
                        cA[0, u, oA:oA + n] = wa
                        cA[1, u, oA:oA + n] = wb
                        cA[2, u, oA:oA + n] = wc
                    elif c == 1:
                        eidx = np.argmax(aa, axis=0)  # the single active edge
                        g = np.arange(n)
                        cA[0, u, oA:oA + n] = wa
                        cA[1, u, oA:oA + n] = wb
                        cA[2, u, oA:oA + n] = wc
                        cB[0, u, oB:oB + n] = np.stack([pen[k][0] for k in range(3)])[eidx, g]
                        cB[1, u, oB:oB + n] = np.stack([pen[k][1] for k in range(3)])[eidx, g]
                        cB[2, u, oB:oB + n] = np.stack([pen[k][2] for k in range(3)])[eidx, g]
                    else:
                        # active edges (2 or 3); order them, dup last if 2
                        P0 = np.stack([pen[k][0] for k in range(3)])  # [3,n]
                        P1 = np.stack([pen[k][1] for k in range(3)])
                        P2 = np.stack([pen[k][2] for k in range(3)])
                        idx = np.argsort(~aa, axis=0, kind='stable')  # actives first
                        ea, ebg, ec = idx[0], idx[1], idx[2]
                        n3 = aa.sum(0) >= 3
                        ec = np.where(n3, ec, ebg)
                        g = np.arange(n)
                        # global halves: A = [Pa-all | W-all], B = [Pb-all | Pc-all]
                        cA[0, u, oA:oA + n] = P0[ea, g]
                        cA[1, u, oA:oA + n] = P1[ea, g]
                        cA[2, u, oA:oA + n] = P2[ea, g]
                        cA[0, u, oA + Tw:oA + Tw + n] = wa
                        cA[1, u, oA + Tw:oA + Tw + n] = wb
                        cA[2, u, oA + Tw:oA + Tw + n] = wc
                        cB[0, u, oB:oB + n] = P0[ebg, g]
                        cB[1, u, oB:oB + n] = P1[ebg, g]
                        cB[2, u, oB:oB + n] = P2[ebg, g]
                        cB[0, u, oB + Tw:oB + Tw + n] = P0[ec, g]
                        cB[1, u, oB + Tw:oB + Tw + n] = P1[ec, g]
                        cB[2, u, oB + Tw:oB + Tw + n] = P2[ec, g]
                    slot_meta.append((b, t))
                core_meta.append((c, rank, slot_meta))
                oA += cap
                if c != 0:
                    oB += cap
        # 3-limb bf16 split -> [72, cols]
        dataA = np.zeros((_K, CA), _BF)
        dataB = np.zeros((_K, CB), _BF)
        for dst, src in ((dataA, cA), (dataB, cB)):
            hi, mid, lo = _split3(src)  # each [3, G, cols]
            for u in range(_G):
                dst[9 * u + 0: 9 * u + 3] = hi[:, u]
                dst[9 * u + 3: 9 * u + 6] = mid[:, u]
                dst[9 * u + 6: 9 * u + 9] = lo[:, u]
        in_maps.append({"A": dataA, "B": dataB})
        meta.append(core_meta)

    # pixel basis [72, 128] block-diagonal (same all cores)
    pix = np.zeros((_K, 128), np.float32)
    dj = np.arange(_PPT, dtype=np.float32) % _T
    di = np.arange(_PPT, dtype=np.float32) // _T
    for u in range(_G):
        for r, row in enumerate((dj, di, np.ones(_PPT, np.float32))):
            for limb in range(3):
                pix[9 * u + 3 * limb + r, _PPT * u: _PPT * (u + 1)] = row
    pix = pix.astype(_BF)
    for i, m in enumerate(in_maps):
        in_maps[i] = {"data": np.concatenate([pix, m["A"], m["B"]], axis=1)}

    layout = dict(cap0=tuple(cap0), cap1=tuple(cap1), capw=tuple(capw),
                  S0=S0, S1=S1, Sw=Sw, T0=T0, T1=T1, Tw=Tw,
                  CA=CA, CB=CB, Stot=(T0 + T1 + Tw) // 4)
    return layout, in_maps, meta


# ------------------------------------------------------------------ device --

def _runs(capl):
    """(offset_cols, offset_slots, cap, count) runs of equal cap."""
    runs = []
    o = 0; s = 0; i = 0
    while i < len(capl):
        j = i
        while j < len(capl) and capl[j] == capl[i]:
            j += 1
        runs.append((o, s, capl[i], j - i))
        o += capl[i] * (j - i)
        s += j - i
        i = j
    return runs


def _build_nc(layout):
    L = layout
    CA, CB, Stot = L["CA"], L["CB"], L["Stot"]
    T0, T1, Tw = L["T0"], L["T1"], L["Tw"]
    S0, S1, Sw = L["S0"], L["S1"], L["Sw"]
    assert CA <= 2048 and CB <= 2048

    nc = bacc.Bacc("TRN2", target_bir_lowering=False, debug=False, num_devices=8)
    TOT = 128 + CA + CB
    data_d = nc.dram_tensor("data", [_K, TOT], _BF16, kind="ExternalInput")
    out_d = nc.dram_tensor("out", [128, Stot], _F32, kind="ExternalOutput")

    def bank_pad(n):
        return -(-n // 512) * 512

    with tile.TileContext(nc) as tc:
        with (
            tc.tile_pool(name="c", bufs=1) as cpool,
            tc.tile_pool(name="s", bufs=1) as spool,
            tc.tile_pool(name="p", bufs=1, space="PSUM") as ppool,
        ):
            # warm the ACT table (overlaps the input-DMA wait)
            warm = spool.tile([128, 1], _BF16, name="warm")
            nc.scalar.copy(warm[:], nc.const_aps.aps[(_F32, 0.0)])

            datat = cpool.tile([_K, TOT], _BF16, name="datat")
            cut = min(128 + CA, TOT)  # pix + all of coefA gate the PA path
            nc.sync.dma_start(datat[:, 0:cut], data_d.ap()[:, 0:cut])
            if cut < TOT:
                nc.sync.dma_start(datat[:, cut:TOT], data_d.ap()[:, cut:TOT])
            pixt = datat[:, 0:128]
            cAt = datat[:, 128:128 + CA]
            cBt = datat[:, 128 + CA:128 + CA + CB]

            PA = ppool.tile([128, bank_pad(CA)], _F32, name="PA")
            PB = ppool.tile([128, bank_pad(CB)], _F32, name="PB")
            for o in range(0, CA, 512):
                e = min(o + 512, CA)
                nc.tensor.matmul(PA[:, o:e], pixt, cAt[:, o:e],
                                 start=True, stop=True)
            for o in range(0, CB, 512):
                e = min(o + 512, CB)
                nc.tensor.matmul(PB[:, o:e], pixt, cBt[:, o:e],
                                 start=True, stop=True)

            # zmin granule: one min per 4 cols; host merges sub-slot minima
            Z0, Z1, Zw = T0 // 4, T1 // 4, Tw // 4
            zmin = spool.tile([128, Stot], _F32, name="zmin")

            # c0: min-reduce W straight from PSUM, one instruction
            if T0:
                v = PA[:, 0:T0].rearrange("p (s w) -> p s w", w=4)
                nc.vector.tensor_reduce(zmin[:, 0:Z0], v,
                                        axis=mybir.AxisListType.X,
                                        op=mybir.AluOpType.min)
                nc.sync.dma_start(out_d.ap()[:, 0:Z0], zmin[:, 0:Z0])

            # ACT copies: cw first (its downstream chain is longer)
            nA = T1 + 2 * Tw
            uA = spool.tile([128, nA], _BF16, name="uA")
            kb = spool.tile([128, nA], _BF16, name="kb")
            keysw = spool.tile([128, max(Tw, 1)], _BF16, name="keysw")
            if Tw:
                nc.scalar.copy(uA[:, T1:nA], PA[:, T0 + T1:T0 + nA])
                nc.vector.tensor_tensor(kb[:, T1:nA], uA[:, T1:nA],
                                        PB[:, T1:nA], op=mybir.AluOpType.max)
                nc.vector.tensor_tensor(keysw[:, 0:Tw], kb[:, T1:T1 + Tw],
                                        kb[:, T1 + Tw:T1 + 2 * Tw],
                                        op=mybir.AluOpType.max)
                k3 = keysw[:, 0:Tw].rearrange("p (s w) -> p s w", w=4)
                nc.vector.tensor_reduce(zmin[:, Z0 + Z1:Z0 + Z1 + Zw], k3,
                                        axis=mybir.AxisListType.X,
                                        op=mybir.AluOpType.min)
                nc.sync.dma_start(out_d.ap()[:, Z0 + Z1:Stot],
                                  zmin[:, Z0 + Z1:Stot])
            if T1:
                nc.scalar.copy(uA[:, 0:T1], PA[:, T0:T0 + T1])
                nc.vector.tensor_tensor(kb[:, 0:T1], uA[:, 0:T1], PB[:, 0:T1],
                                        op=mybir.AluOpType.max)
                v = kb[:, 0:T1].rearrange("p (s w) -> p s w", w=4)
                nc.vector.tensor_reduce(zmin[:, Z0:Z0 + Z1], v,
                                        axis=mybir.AxisListType.X,
                                        op=mybir.AluOpType.min)
                nc.scalar.dma_start(out_d.ap()[:, Z0:Z0 + Z1],
                                    zmin[:, Z0:Z0 + Z1])

    nc.compile()
    return nc


def _get_nc(layout):
    key = (layout["cap0"], layout["cap1"], layout["capw"])
    if key not in _NC_CACHE:
        _NC_CACHE[key] = _build_nc(layout)
    return _NC_CACHE[key]


# ------------------------------------------------------------------ kernel --

def kernel(vertices, faces):
    vertices = np.asarray(vertices)
    faces = np.asarray(faces)
    layout, in_maps, meta = _prepare(vertices, faces)
    nc = _get_nc(layout)
    kw = dict(PROFILE.get("run_kwargs", {}))
    res = run_bass_kernel_spmd(nc, in_maps, list(range(8)), **kw)
    PROFILE["last_result"] = res

    out = np.full((_B, _HW, _HW), _CLAMP, np.float32)
    caps = {0: layout["cap0"], 1: layout["cap1"], 2: layout["capw"]}
    Z0, Z1 = layout["T0"] // 4, layout["T1"] // 4
    zbase = {0: 0, 1: Z0, 2: Z0 + Z1}
    for core in range(8):
        z = res.results[core]["out"]  # [128, Stot]
        cum = {0: 0, 1: 0, 2: 0}
        for c, rank, slot_meta in meta[core]:
            ncol = caps[c][rank] // 4
            col = zbase[c] + cum[c]
            cum[c] += ncol
            if not slot_meta:
                continue
            smin = z[:, col:col + ncol].min(axis=1)
            for u, (b, t) in enumerate(slot_meta):
                blk = smin[_PPT * u:_PPT * (u + 1)].reshape(_T, _T)
                tjj = (t % _NTJ) * _T
                tii = (t // _NTJ) * _T
                out[b, tii:tii + _T, tjj:tjj + _T] = np.minimum(
                    out[b, tii:tii + _T, tjj:tjj + _T], blk)
    return out
